# revision 74
# baseline (speedup 1.0000x reference)
"""Causal multi-head attention (B=4, T=2048, D=1024, H=16) on 8 NeuronCores.

Sharding:
  stage 1 (QKV proj + attention): core c -> batch c//2, head-group c%2
    (8 of 16 heads, 512 of 1024 channels). Data-parallel on B, tensor-
    parallel on heads.
  stage 2 (output projection): one 8-rank AllToAll re-shards attention
    output to (all 4 batches x 256-token t-slice) per core, then each core
    computes out = attn_out @ W_O.T for its 1024 rows. No reduction needed.

Mixed precision (chosen against the 2e-2 rel-err budget, measured offline on
the fixed inputs):
  - fp8e4 DoubleRow matmuls (0.5 PE cycles/row, 256-wide contraction) carry
    the bulk: QKV projections for token chunks 1-3, QK for q-chunks 1-3, and
    the m=1 half of the output projection. Host pre-quantizes x/W to fp8 and
    pre-arranges the [128p, pair, row, .] DoubleRow layouts.
  - fp16 carries the precision-critical paths: chunk-0 projections, qc0
    attention, all probabilities (fp8 p underflows: softmax mass sits ~1e-4
    relative to max, below fp8's 2^-9 floor), V, and the m=0 output
    projection, which protects the small-neff early-token rows.
  - Q/K fp8 tiles are produced by a PSUM->SBUF fp8 copy plus 4 partition-fold
    SBUF->SBUF DMAs per chunk into the [32, 2, .] DoubleRow layout.

exp runs on ACT reading PSUM with the softmax scale fused, writing fp16; the
denominator comes free as a 65th output row of the PV matmul (V augmented
with a ones column). Causal masking multiplies only the 128-wide diagonal
square by one shared triangle mask.
"""
import numpy as np
import ml_dtypes

import concourse.bass as bass
import concourse.mybir as mybir
import concourse.tile as tile
from concourse.bass_utils import run_bass_kernel_spmd

F32 = mybir.dt.float32
F32R = mybir.dt.float32r
F16 = mybir.dt.float16
F8 = mybir.dt.float8e4
NPF8 = ml_dtypes.float8_e4m3
DRM = mybir.MatmulPerfMode.DoubleRow

P = 128
B, T, D = 4, 2048, 1024
H, HD = 16, 64
NCORES = 8
CH = D // 2          # channels per core (8 heads)
NHP = 4              # head pairs per core
NKT = T // P         # 16 k-tiles
NQC = T // 512       # 4 q-chunks
NIT = D // P         # 8 input-dim tiles
NA = 4               # DoubleRow 256-wide contraction steps over D


def _split_multiwaits(nc) -> int:
    """walrus here rejects >1 sem wait per instruction; split extras into
    wait-only NoOps on the same engine."""
    nsplit = 0
    for f in nc.m.functions:
        for bb in f.blocks:
            if not any(
                i.sync_info is not None and i.sync_info.on_wait is not None
                and len(i.sync_info.on_wait) > 1 for i in bb.instructions
            ):
                continue
            new_list = []
            for inst in bb.instructions:
                si = inst.sync_info
                if si is not None and si.on_wait is not None and len(si.on_wait) > 1:
                    waits = list(si.on_wait)
                    for k, w in enumerate(waits[:-1]):
                        n = mybir.InstNoOp(
                            name=f"{inst.name}-wsplit{k}", ins=[], outs=[])
                        n.engine = inst.engine
                        n.sync_info = mybir.SyncInfo(on_wait=[w], on_update=[])
                        new_list.append(n)
                        nsplit += 1
                    inst.sync_info = mybir.SyncInfo(
                        on_wait=[waits[-1]], on_update=list(si.on_update or []))
                new_list.append(inst)
            bb.instructions = new_list
    return nsplit


def _build_nc(sim: bool = False):
    nc = bass.Bass("TRN2", target_bir_lowering=False, debug=False,
                   num_devices=NCORES)
    x16_d = nc.dram_tensor("x16", [P, NIT, 512], F16, kind="ExternalInput").ap()
    x8_d = nc.dram_tensor("x8", [P, NA, 2, 1536], F8, kind="ExternalInput").ap()
    wq16_d = nc.dram_tensor("wq16", [P, NIT, CH], F16, kind="ExternalInput").ap()
    wk16_d = nc.dram_tensor("wk16", [P, NIT, CH], F16, kind="ExternalInput").ap()
    wv16_d = nc.dram_tensor("wv16", [P, NIT, CH], F16, kind="ExternalInput").ap()
    wq8_d = nc.dram_tensor("wq8", [P, NA, 2, CH], F8, kind="ExternalInput").ap()
    wk8_d = nc.dram_tensor("wk8", [P, NA, 2, CH], F8, kind="ExternalInput").ap()
    wv8_d = nc.dram_tensor("wv8", [P, NA, 2, CH], F8, kind="ExternalInput").ap()
    wo16_d = nc.dram_tensor("wo16", [P, NIT, D], F16, kind="ExternalInput").ap()
    wo8_d = nc.dram_tensor("wo8", [P, NA, 2, D], F8, kind="ExternalInput").ap()
    ones_d = nc.dram_tensor("ones", [P, 64], F32R, kind="ExternalInput").ap()
    out_d = nc.dram_tensor("out", [B, 2, P, D], F16,
                           kind="ExternalOutput").ap()
    a2a_in0 = nc.dram_tensor("a2a_in0", [NCORES, CH, P], F16).ap()
    a2a_out0 = nc.dram_tensor("a2a_out0", [NCORES, CH, P], F16).ap()
    # m=1 is split into two 64-row bands (qc2 half / qc3 half) so the qc2
    # collective overlaps qc3's attention
    a2a_in1a = nc.dram_tensor("a2a_in1a", [NCORES, CH, 64], F8).ap()
    a2a_out1a = nc.dram_tensor("a2a_out1a", [NCORES, CH, 64], F8).ap()
    # the qc3 band ships per head-pair: each 64KB collective fires as soon
    # as its head-pair's tail completes, overlapping the remaining attention
    a2a_in1b = nc.dram_tensor("a2a_in1b", [NHP, NCORES, P, 64], F8).ap()
    a2a_out1b = nc.dram_tensor("a2a_out1b", [NHP, NCORES, P, 64], F8).ap()

    scale = float(1.0 / np.sqrt(HD))

    with tile.TileContext(nc) as tc:
        with (
            tc.tile_pool(name="persist", bufs=1) as persist,
        ):
            # ---- persistent SBUF tensors -------------------------------
            # dependency tracking is whole-tile: K^T/V live in per-chunk
            # tiles so a later chunk's projection writes never serialize an
            # earlier q-chunk's attention reads
            kt16 = persist.tile([P, NHP, 512], F16)        # K^T tiles 0:4
            kt8c = [persist.tile([32, 2, NHP, 2, 512], F8, name=f"kt8c{c}")
                    for c in range(NQC)]
            vac = [persist.tile([P, 4, NHP, 2, HD + 1], F16, name=f"vac{c}")
                   for c in range(NQC)]

            with (
                tc.tile_pool(name="wpool", bufs=1) as wpool,
                tc.tile_pool(name="w8pool", bufs=1) as w8pool,
                tc.tile_pool(name="xpool", bufs=1) as xpool,
                tc.tile_pool(name="stg_pool", bufs=2) as stg_pool,
                tc.tile_pool(name="ob_pool", bufs=4) as ob_pool,
                tc.tile_pool(name="osb_pool", bufs=2) as osb_pool,
                tc.tile_pool(name="qpool", bufs=2) as qpool,
                tc.tile_pool(name="q8pool", bufs=2) as q8pool,
                tc.tile_pool(name="ao_pool", bufs=2) as ao_pool,
                tc.tile_pool(name="mpool", bufs=1) as mpool,
                tc.tile_pool(name="pt_pool", bufs=4) as pt_pool,
                tc.tile_pool(name="nrm_pool", bufs=1) as nrm_pool,
                tc.tile_pool(name="ppool", bufs=2, space="PSUM") as ppool,
                tc.tile_pool(name="ps_s", bufs=2, space="PSUM") as ps_s,
                tc.tile_pool(name="ps_pv", bufs=1, space="PSUM") as ps_pv,
            ):
                # chunk-0 fp16 operands live as half-tiles (its 0-3 / 4-7)
                # so the first projection half-groups only depend on the
                # first half-transfers (dependency tracking is whole-tile)
                w16 = {
                    "q": [wpool.tile([P, 4, CH], F16, tag=f"wq{h}",
                                     name=f"w16q{h}") for h in range(2)],
                    "k": [wpool.tile([P, 4, CH], F16, tag=f"wk{h}",
                                     name=f"w16k{h}") for h in range(2)],
                    "v": [wpool.tile([P, 4, CH], F16, tag=f"wv{h}",
                                     name=f"w16v{h}") for h in range(2)],
                }
                w8 = {
                    "q": w8pool.tile([P, NA, 2, CH], F8, tag="wq8",
                                     name="w8q"),
                    "k": w8pool.tile([P, NA, 2, CH], F8, tag="wk8",
                                     name="w8k"),
                    "v": w8pool.tile([P, NA, 2, CH], F8, tag="wv8",
                                     name="w8v"),
                }
                x16 = [xpool.tile([P, 4, 512], F16, tag=f"x16{h}",
                                  name=f"x16{h}") for h in range(2)]
                x8 = xpool.tile([P, NA, 2, 1536], F8, tag="x8")
                # batched DMAs (each costs ~625ns serialized HWDGE trigger
                # overhead), but the chunk-0 operands load in halves: the
                # first projection half-groups start after half a transfer
                for h in range(2):
                    nc.sync.dma_start(x16[h][:], x16_d[:, 4 * h:4 * h + 4])
                    nc.sync.dma_start(w16["v"][h][:],
                                      wv16_d[:, 4 * h:4 * h + 4])
                    nc.sync.dma_start(w16["q"][h][:],
                                      wq16_d[:, 4 * h:4 * h + 4])
                    nc.sync.dma_start(w16["k"][h][:],
                                      wk16_d[:, 4 * h:4 * h + 4])
                nc.sync.dma_start(x8[:], x8_d)
                for name, d in (("q", wq8_d), ("k", wk8_d), ("v", wv8_d)):
                    nc.sync.dma_start(w8[name][:], d)

                ones64 = mpool.tile([P, 64], F32R)
                nc.sync.dma_start(ones64[:], ones_d)
                # fill the V|ones denominator columns via broadcast copies
                for c in range(NQC):
                    nc.scalar.copy(
                        vac[c][:, :, :, :, HD],
                        ones64[:, 0:1].to_broadcast((P, 4, NHP, 2)))
                # one shared triangle mask for the 128-wide diagonal square:
                # within a diagonal block, q_local >= k_local is visible
                trimask = mpool.tile([P, P], F16, tag="trimask")
                nc.gpsimd.memset(trimask[:], 1.0)
                nc.gpsimd.affine_select(
                    out=trimask[:], in_=trimask[:],
                    compare_op=mybir.AluOpType.is_ge,
                    fill=0.0, base=0, channel_multiplier=-1,
                    pattern=[[1, P]])

                pending = []
                normtail = []
                filler_acc = [0.0]

                def emit_fillers(remaining_units, pace=1.0):
                    if not pending:
                        return
                    filler_acc[0] += len(pending) / max(1.0,
                                                        pace * remaining_units)
                    while filler_acc[0] >= 1.0 and pending:
                        filler_acc[0] -= 1.0
                        pending.pop(0)()

                def kfold(stage, chunk):
                    """4 partition-fold DMAs [128,NHP,512]->[32,2,NHP,2,512]"""
                    def g():
                        for hh in range(2):
                            for jj in range(2):
                                nc.sync.dma_start(
                                    kt8c[chunk][:, jj, :, hh],
                                    stage[hh * 64 + jj * 32:
                                          hh * 64 + jj * 32 + 32])
                    return g

                def qfold(stage, q8):
                    def g():
                        for hh in range(2):
                            for jj in range(2):
                                nc.sync.dma_start(
                                    q8[:, jj, :, hh],
                                    stage[hh * 64 + jj * 32:
                                          hh * 64 + jj * 32 + 32])
                    return g

                def project16():
                    """chunk-0 fp16 projections, interleaved with qc0: only
                    [v0, q0, k0] must precede hp0; the rest pace through
                    qc0's attention windows (ACT-bound) as PE fillers."""
                    qtc = qpool.tile([P, NHP, 512], F16, tag="qtc")
                    kstage = stg_pool.tile([P, NHP, 512], F8, tag="kstage")

                    # fp16 groups are emitted as two half-closures so one
                    # filler pop costs PE ~0.85us, not 1.7us (chunky fillers
                    # otherwise delay QK and starve ACT)
                    def qk_group(w, ot, dst_f16, stage, half, box):
                        def g():
                            if half == 0:
                                box["ps"] = ppool.tile([P, 512], F32,
                                                       tag="proj",
                                                       name="qkg_ps")
                            ps = box["ps"]
                            for it in range(4):
                                nc.tensor.matmul(
                                    ps[:],
                                    w[half][:, it, ot * P:(ot + 1) * P],
                                    x16[half][:, it],
                                    start=(half == 0 and it == 0),
                                    stop=(half == 1 and it == 3))
                            if half == 1:
                                nc.vector.tensor_copy(dst_f16[:, ot], ps[:])
                                if stage is not None:
                                    nc.vector.tensor_copy(stage[:, ot],
                                                          ps[:])
                        return g

                    def vg2(tt):
                        box = {}

                        def mk(half):
                            def g():
                                if half == 0:
                                    box["ps"] = ppool.tile([P, 512], F32,
                                                           tag="proj",
                                                           name="vg_ps")
                                ps = box["ps"]
                                for it in range(4):
                                    nc.tensor.matmul(
                                        ps[:],
                                        x16[half][:, it, tt * P:(tt + 1) * P],
                                        w16["v"][half][:, it],
                                        start=(half == 0 and it == 0),
                                        stop=(half == 1 and it == 3))
                                if half == 1:
                                    nc.vector.tensor_copy(
                                        vac[0][:, tt, :, :, 0:HD],
                                        ps[:].rearrange(
                                            "p (hp h d) -> p hp h d",
                                            hp=NHP, h=2))
                            return g
                        return mk(0), mk(1)

                    def qkg2(w, ot, dst_f16, stage):
                        box = {}
                        return (qk_group(w, ot, dst_f16, stage, 0, box),
                                qk_group(w, ot, dst_f16, stage, 1, box))

                    pending.extend(vg2(0))
                    pending.extend(qkg2(w16["q"], 0, qtc, None))
                    pending.extend(qkg2(w16["k"], 0, kt16, kstage))
                    for tt in range(1, 4):
                        pending.extend(vg2(tt))
                    for ot in range(1, NHP):
                        pending.extend(qkg2(w16["q"], ot, qtc, None))
                        pending.extend(qkg2(w16["k"], ot, kt16, kstage))
                    pending.append(kfold(kstage, 0))
                    return qtc

                def project8(tc4):
                    """fp8 DoubleRow projections for chunk tc4 (1..3)."""
                    tsl = slice((tc4 - 1) * 512, tc4 * 512)
                    qstage = stg_pool.tile([P, NHP, 512], F8, tag="qstage")
                    kstage = stg_pool.tile([P, NHP, 512], F8, tag="kstage")
                    q8 = q8pool.tile([32, 2, NHP, 2, 512], F8, tag="q8")

                    def qk_group(w, ot, stage):
                        def g():
                            ps = ppool.tile([P, 512], F32, tag="proj")
                            for a in range(NA):
                                nc.tensor.matmul(
                                    ps[:], w[:, a, :, ot * P:(ot + 1) * P],
                                    x8[:, a, :, tsl], start=(a == 0),
                                    stop=(a == NA - 1), perf_mode=DRM)
                            nc.vector.tensor_copy(stage[:, ot], ps[:])
                        return g

                    def v_group(tt):
                        def g():
                            ps = ppool.tile([P, 512], F32, tag="proj")
                            t0 = (tc4 - 1) * 512 + tt * P
                            for a in range(NA):
                                nc.tensor.matmul(
                                    ps[:], x8[:, a, :, t0:t0 + P],
                                    w8["v"][:, a], start=(a == 0),
                                    stop=(a == NA - 1), perf_mode=DRM)
                            nc.vector.tensor_copy(
                                vac[tc4][:, tt, :, :, 0:HD],
                                ps[:].rearrange("p (hp h d) -> p hp h d",
                                                hp=NHP, h=2))
                        return g

                    for ot in range(NHP):
                        pending.append(qk_group(w8["q"], ot, qstage))
                    pending.append(qfold(qstage, q8))
                    for ot in range(NHP):
                        pending.append(qk_group(w8["k"], ot, kstage))
                    pending.append(kfold(kstage, tc4))
                    for tt in range(4):
                        pending.append(v_group(tt))
                    return q8

                def attend(hp, qc, qsrc, aoq, ao_f8):
                    """Attention for head-pair hp, q-chunk qc. qc0 runs the
                    fp16 2-matmul QK; qc1-3 run fp8 DoubleRow QK. kt loop is
                    software-pipelined: QK(kt+1) issues before PV(kt)."""
                    nkt = 4 * (qc + 1)
                    pva = ps_pv.tile([HD + 1, 512], F32, tag="pva")
                    pvb = ps_pv.tile([HD + 1, 512], F32, tag="pvb")
                    s2s = {}

                    def qk(kt):
                        ksl = slice(kt * P, (kt + 1) * P)
                        f0 = max(0, kt - 4 * qc) * P
                        s2 = ps_s.tile([P, 1024], F32, tag="s2")
                        if qc == 0:
                            nc.tensor.matmul(s2[:, f0:512],
                                             kt16[0:64, hp, ksl],
                                             qsrc[0:64, hp, f0:],
                                             start=True, stop=True)
                            nc.tensor.matmul(s2[:, 512 + f0:1024],
                                             kt16[64:128, hp, ksl],
                                             qsrc[64:128, hp, f0:],
                                             start=True, stop=True)
                        else:
                            for h in range(2):
                                nc.tensor.matmul(
                                    s2[:, h * 512 + f0:(h + 1) * 512],
                                    kt8c[kt // 4][:, :, hp, h,
                                                  (kt % 4) * P:
                                                  (kt % 4 + 1) * P],
                                    qsrc[:, :, hp, h, f0:],
                                    start=True, stop=True, perf_mode=DRM)
                        s2s[kt] = s2

                    def softmax_pv(kt):
                        s2 = s2s.pop(kt)
                        pt = pt_pool.tile([P, 2, 512], F16, tag="pt")
                        di = kt - 4 * qc
                        f0 = max(0, di) * P
                        if f0 > 0:
                            s2v = s2[:].rearrange("p (a b) -> p a b", a=2)
                            nc.scalar.activation(
                                pt[:, :, f0:], s2v[:, :, f0:],
                                mybir.ActivationFunctionType.Exp,
                                scale=scale)
                        else:
                            nc.scalar.activation(
                                pt[:].rearrange("p a b -> p (a b)"), s2[:],
                                mybir.ActivationFunctionType.Exp,
                                scale=scale)
                        if di >= 0:
                            # mask only the 128-wide diagonal square, on the
                            # otherwise-idle Pool engine (keeps DVE free and
                            # dodges its queue)
                            nc.gpsimd.tensor_mul(
                                pt[:, :, f0:f0 + P], pt[:, :, f0:f0 + P],
                                trimask[:, None, :].to_broadcast((P, 2, P)))
                        vat = vac[kt // 4]
                        nc.tensor.matmul(pva[:, f0:],
                                         vat[:, kt % 4, hp, 0],
                                         pt[:, 0, f0:],
                                         start=(kt == 0), stop=(kt == nkt - 1))
                        nc.tensor.matmul(pvb[:, f0:],
                                         vat[:, kt % 4, hp, 1],
                                         pt[:, 1, f0:],
                                         start=(kt == 0), stop=(kt == nkt - 1))
                        if kt >= 2 and normtail:
                            t, post = normtail.pop(0)
                            t()
                            if post is not None:
                                post()

                    # qc3 has the deepest filler queue (m=0 and band-A m=1
                    # output projections) and no phase after it: drain faster
                    pace = 1.0 if qc == 3 else 0.8
                    qk(0)
                    for kt in range(1, nkt):
                        qk(kt)
                        # fillers go between QK(kt) and PV(kt-1) in the PE
                        # queue: PV blocks on exp (ACT is the loop pacer), so
                        # PE runs projection groups during that wait instead
                        # of head-of-line stalling
                        emit_fillers((nkt - kt) + (NHP - 1 - hp) * nkt, pace)
                        softmax_pv(kt - 1)
                    emit_fillers(1 + (NHP - 1 - hp) * nkt, pace)
                    softmax_pv(nkt - 1)

                    # copy PV accumulators out of PSUM fast (frees banks);
                    # defer the recip->broadcast->scale tail
                    pvs = nrm_pool.tile([P, 2, 512], F32, tag="pvs")
                    nc.vector.tensor_copy(pvs[0:65, 0], pva[:])
                    if (hp == NHP - 1 and qc in (1, 2)) or qc == 3:
                        # this tail gates a collective launch: split the two
                        # copies across DVE/ACT to shorten the serial chain
                        nc.scalar.copy(pvs[0:65, 1], pvb[:])
                    else:
                        nc.vector.tensor_copy(pvs[0:65, 1], pvb[:])
                    rden = nrm_pool.tile([P, 2, 512], F32R, tag="rden")
                    with nc.allow_low_precision("f32r softmax denominators"):
                        nc.vector.reciprocal(rden[64:65, 0], pvs[64:65, 0])
                        nc.vector.reciprocal(rden[64:65, 1], pvs[64:65, 1])

                    def tail(hp=hp, qc=qc, pvs=pvs, rden=rden, aoq=aoq,
                             ao_f8=ao_f8):
                        # broadcast 1/den across partitions via tiny PE
                        # matmuls (walrus rejects InstPartitionBroadcast)
                        rba = ppool.tile([64, 512], F32, tag="proj")
                        rbb = ppool.tile([64, 512], F32, tag="proj")
                        nc.tensor.matmul(rba[:], ones64[64:65, :],
                                         rden[64:65, 0], start=True, stop=True)
                        nc.tensor.matmul(rbb[:], ones64[64:65, :],
                                         rden[64:65, 1], start=True, stop=True)
                        nc.vector.tensor_mul(aoq[0:64, hp], pvs[0:64, 0],
                                             rba[:])
                        nc.vector.tensor_mul(aoq[64:128, hp], pvs[0:64, 1],
                                             rbb[:])
                        if ao_f8:
                            a2a = a2a_r1a[:, hp] if qc == 2 else a2a_r1b[:, hp]
                            nc.sync.dma_start(
                                a2a,
                                aoq[:, hp].rearrange("p (j t) -> p j t", j=8))
                        else:
                            nc.sync.dma_start(
                                a2a_r0[:, hp, (qc % 2) * 4:(qc % 2) * 4 + 4],
                                aoq[:, hp].rearrange("p (j t) -> p j t", j=4))
                    if qc == 3:
                        def post(hp=hp):
                            emit_collective(a2a_in1b[hp], a2a_out1b[hp])
                            # this head-pair's band-B input loads as soon as
                            # its collective lands
                            nc.sync.dma_start(
                                aob_hp[hp][:],
                                a2a_out1b[hp].rearrange(
                                    "(b sg) p t -> p b sg t", sg=2))
                        normtail.append((tail, post))
                    else:
                        normtail.append((tail, None))

                a2a_r0 = a2a_in0.rearrange("j (hp p) t -> p hp j t", p=P)
                a2a_r1a = a2a_in1a.rearrange("j (hp p) t -> p hp j t", p=P)
                a2a_r1b = a2a_in1b.rearrange("hp j p t -> p hp j t")

                def emit_collective(cin, cout):
                    if sim:
                        nc.sync.dma_start(cout, cin)
                    else:
                        nc.gpsimd.collective_compute(
                            "AllToAll", mybir.AluOpType.bypass,
                            replica_groups=[list(range(NCORES))],
                            ins=[cin], outs=[cout])

                # W_O tiles reuse the fp16 W half-slots, which are only
                # read by chunk-0 projections; prefetch during qc1
                wo16a = [wpool.tile([P, 4, 512], F16, tag=f"wq{h}",
                                    name=f"wo16a{h}") for h in range(2)]
                wo16b = [wpool.tile([P, 4, 512], F16, tag=f"wk{h}",
                                    name=f"wo16b{h}") for h in range(2)]
                wo8t_h = [wpool.tile([P, 2, 2, D], F8, tag=f"wv{h}",
                                     name=f"wo8t{h}") for h in range(2)]

                def wo8t(a):
                    return wo8t_h[a // 2][:, a % 2]

                def wo_dma():
                    def g():
                        for h in range(2):
                            nc.sync.dma_start(
                                wo16a[h][:],
                                wo16_d[:, 4 * h:4 * h + 4, 0:512])
                            nc.sync.dma_start(
                                wo16b[h][:],
                                wo16_d[:, 4 * h:4 * h + 4, 512:1024])
                            nc.sync.dma_start(wo8t_h[h][:],
                                              wo8_d[:, 2 * h:2 * h + 2])
                    return g

                def o_group16(b, aob, osb, w, oc):
                    def g():
                        ps = ppool.tile([P, 512], F32, tag="proj")
                        for ct in range(NIT):
                            nc.tensor.matmul(
                                ps[:], aob[:, ct], w[ct // 4][:, ct % 4],
                                start=(ct == 0), stop=(ct == NIT - 1))
                        nc.vector.tensor_copy(
                            osb[:, oc * 512:(oc + 1) * 512], ps[:])
                        if oc == 1:
                            nc.sync.dma_start(out_d[b, 0], osb[:])
                    return g

                def o_load16(b):
                    """eager aob load; the matmul groups go to pending"""
                    aob = ob_pool.tile([P, NIT, P], F16, tag="aob16")
                    osb = osb_pool.tile([P, D], F16, tag="osb")
                    nc.sync.dma_start(
                        aob[:],
                        a2a_out0[2 * b:2 * b + 2].rearrange(
                            "s (c p) t -> p (s c) t", p=P))
                    pending.append(o_group16(b, aob, osb, wo16a, 0))
                    pending.append(o_group16(b, aob, osb, wo16b, 1))

                def o_group8(b, aob, osb, oc, band):
                    def g():
                        ps = ppool.tile([64, 512], F32, tag="proj",
                                        name="ps8")
                        for a in range(NA):
                            nc.tensor.matmul(
                                ps[:], aob[:, a],
                                wo8t(a)[:, :, oc * 512:(oc + 1) * 512],
                                start=(a == 0), stop=(a == NA - 1),
                                perf_mode=DRM)
                        nc.vector.tensor_copy(
                            osb[:, oc * 512:(oc + 1) * 512], ps[:])
                        if oc == 1:
                            nc.sync.dma_start(
                                out_d[b, 1, band * 64:(band + 1) * 64],
                                osb[:])
                    return g

                def o_load8(b, cout, band):
                    """one 64-row band of the m=1 fp8 DoubleRow projection"""
                    aob = ob_pool.tile([P, NA, 2, 64], F8, tag="aob8")
                    osb = osb_pool.tile([64, D], F16, tag="osb8")
                    for sg in range(2):
                        nc.sync.dma_start(
                            aob[:, :, sg],
                            cout[2 * b + sg].rearrange(
                                "(c p) t -> p c t", p=P))
                    pending.append(o_group8(b, aob, osb, 0, band))
                    pending.append(o_group8(b, aob, osb, 1, band))

                # band-B input, one tile per head-pair (whole-tile dependency
                # tracking: a shared tile would serialize the first band-B
                # matmul on the LAST head-pair's collective). wo8's channel
                # blocks are host-ordered (hp, sg) so DR step a reads only
                # head-pair a's tile.
                aob_hp = [ob_pool.tile([P, B, 2, 64], F8, name=f"aobhp{h}")
                          for h in range(NHP)]

                qsrc = project16()
                while pending:
                    pending.pop(0)()
                for tc4 in range(NQC):
                    if tc4 + 1 < NQC:
                        next_qsrc = project8(tc4 + 1)
                    if tc4 == 1:
                        pending.append(wo_dma())
                    if tc4 == 3:
                        # qc2's band of m=1: a2a_out1a lands early in qc3;
                        # the loads wait on the collective, the matmul groups
                        # pace through qc3
                        for b in range(B):
                            o_load8(b, a2a_out1a, 0)
                    ao_f8 = tc4 >= 2
                    if ao_f8:
                        aoq = ao_pool.tile([P, NHP, 512], F8, tag="ao8")
                    else:
                        aoq = ao_pool.tile([P, NHP, 512], F16, tag="ao16")
                    for hp in range(NHP):
                        attend(hp, tc4, qsrc, aoq, ao_f8)
                    if tc4 in (1, 2):
                        # tails gate the collective inputs; pending keeps
                        # pacing into the next phase
                        while normtail:
                            t, post = normtail.pop(0)
                            t()
                    if tc4 == 1:
                        emit_collective(a2a_in0, a2a_out0)
                    if tc4 == 2:
                        emit_collective(a2a_in1a, a2a_out1a)
                        # m=0 output projection fills qc3 (balance: qc3 has
                        # the most ACT work and no projection fillers left)
                        for b in range(B):
                            o_load16(b)
                    if tc4 + 1 < NQC:
                        qsrc = next_qsrc
                while normtail:
                    t, post = normtail.pop(0)
                    t()
                    if post is not None:
                        post()
                while pending:
                    pending.pop(0)()

                # ---- qc3's band of m=1 (fp8 DoubleRow tail) ------------
                for b in range(B):
                    osb = osb_pool.tile([64, D], F16, tag="osb8")
                    for oc in range(2):
                        # attention PSUM pools are idle by now: spread the
                        # band-B groups over both so more groups' early
                        # steps run while the last head-pair's data flies
                        pool = ppool if (2 * b + oc) % 2 == 0 else ps_s
                        ps = pool.tile([64, 512], F32, tag="proj"
                                       if pool is ppool else "s2",
                                       name="psb")
                        # DR step a contracts head-pair a's channels; steps
                        # run in head-pair completion order
                        for a in range(NA):
                            nc.tensor.matmul(
                                ps[:],
                                aob_hp[a][:, b],
                                wo8t(a)[:, :, oc * 512:(oc + 1) * 512],
                                start=(a == 0), stop=(a == NA - 1),
                                perf_mode=DRM)
                        nc.vector.tensor_copy(
                            osb[:, oc * 512:(oc + 1) * 512], ps[:])
                    nc.sync.dma_start(out_d[b, 1, 64:128], osb[:])

    _split_multiwaits(nc)
    return nc


_NC_CACHE = None


def _get_nc():
    global _NC_CACHE
    if _NC_CACHE is None:
        _NC_CACHE = _build_nc()
    return _NC_CACHE


def make_in_maps(x, W_Q, W_K, W_V, W_O):
    x = np.asarray(x, np.float32)
    wqt = np.ascontiguousarray(np.asarray(W_Q, np.float32).T)
    wkt = np.ascontiguousarray(np.asarray(W_K, np.float32).T)
    wvt = np.ascontiguousarray(np.asarray(W_V, np.float32).T)
    wot = np.ascontiguousarray(np.asarray(W_O, np.float32).T)
    ones = np.ones((P, 64), np.float32)

    def w_lay16(w):  # [1024, 512] -> [128, 8, 512] fp16
        return np.ascontiguousarray(
            w.reshape(NIT, P, -1).transpose(1, 0, 2).astype(np.float16))

    def w_lay8(w):  # [1024, N] -> [128, 4, 2, N] fp8
        return np.ascontiguousarray(
            w.reshape(NA, 2, P, -1).transpose(2, 0, 1, 3).astype(NPF8))

    wo16 = w_lay16(wot)  # [128, 8, 1024]
    # wo8 block (a, jj) holds channel jj*512 + a*128 + p: DR step a then
    # contracts exactly head-pair a's channels of both source groups
    wo8 = np.ascontiguousarray(
        wot.reshape(2, NA, P, -1).transpose(2, 1, 0, 3).astype(NPF8))
    in_maps = []
    for c in range(NCORES):
        b, g = c // 2, c % 2
        xb = x[b].T  # [1024, 2048]
        cols = slice(g * CH, (g + 1) * CH)
        in_maps.append({
            "x16": w_lay16(xb[:, 0:512]),
            "x8": w_lay8(xb[:, 512:2048]),
            "wq16": w_lay16(wqt[:, cols]),
            "wk16": w_lay16(wkt[:, cols]),
            "wv16": w_lay16(wvt[:, cols]),
            "wq8": w_lay8(wqt[:, cols]),
            "wk8": w_lay8(wkt[:, cols]),
            "wv8": w_lay8(wvt[:, cols]),
            "wo16": wo16,
            "wo8": wo8,
            "ones": ones,
        })
    return in_maps


def assemble(results):
    out = np.empty((B, T, D), np.float32)
    for j in range(NCORES):
        o = results[j]["out"]  # [B, 2, 128, D]
        for b in range(B):
            out[b, j * P:(j + 1) * P, :] = o[b, 0]
            # m=1 is two 64-row bands: qc2's half and qc3's half
            out[b, 1024 + j * 64:1024 + (j + 1) * 64, :] = o[b, 1, 0:64]
            out[b, 1536 + j * 64:1536 + (j + 1) * 64, :] = o[b, 1, 64:128]
    return out


def kernel(x, W_Q, W_K, W_V, W_O):
    in_maps = make_in_maps(x, W_Q, W_K, W_V, W_O)
    nc = _get_nc()
    res = run_bass_kernel_spmd(nc, in_maps, core_ids=list(range(NCORES)))
    return assemble(res.results)
